# revision 16
# baseline (speedup 1.0000x reference)
"""Trainium2 Bass kernel for the CNN flow layer (dilated conv1d + leaky-relu
+ per-feature scale + skip connection, with per-row logdet).

Math (per row b of x[B, DIM]):
    c[i]   = sum_k w[k] * x[i + 2k] + bias      (right-zero-padded, K=7, dil=2)
    act    = lrelu(c, 0.01)
    out    = act * scale + x                    (scale per-feature, from w0/lmbd)
    logdet = sum_i log|g*scale*w0 + 1|,  g = 1 if c>=0 else 0.01

Strategy (pure data parallel over 8 cores, natural layout rows-on-partitions):
  - conv on TensorE: 7 accumulated matmuls per PSUM bank with scaled-identity
    stationary operands (lhsT = w_k * I_128), rhs = shifted x slices (fp32r,
    full rate at N=512). Keeps rows on partitions — no transposes — and moves
    the 7-tap MAC off the vector engine.
  - ACT: pad memzero, Lrelu(conv+bias) from PSUM -> fp16 act.
  - DVE: t = act*scale_b (fp16 TT 2x), out_f32 = (t*2^z) + x_f32 (STT),
    logdet tile contribution = accum_out of (act >= 0) * d_b (fused STT).
  - logdet epilogue: PE transpose of the [128, 16] per-tile accum columns,
    bias-add of sum(lb) on ACT, single DMA out.
  - per-feature constants (scale, d = la-lb, identity packs) are computed on
    host from the runtime weight/bias/lmbd values and fed as extra inputs,
    so the compiled NEFF is value-independent.
"""

import math
from contextlib import ExitStack

import numpy as np

N_CORES = 8
BATCH = 16384
DIM = 1024
KTAPS = 7
DIL = 2
PAD = (KTAPS - 1) * DIL  # 12
DIMP = DIM + PAD
NEG = 0.01
R = BATCH // N_CORES  # rows per core (2048)
P = 128
NT = R // P  # row tiles per core (16)
SLAB = 4  # row tiles per DMA slab
NS = NT // SLAB
NBLK = DIM // 512  # 512-col psum blocks per tile

_PROGRAM_CACHE = {}


def _build_program(sim_compat=False):
    import concourse.bacc as bacc
    import concourse.bass as bass
    import concourse.mybir as mybir
    import concourse.tile as tile

    f32 = mybir.dt.float32
    f32r = mybir.dt.float32r
    f16 = mybir.dt.float16

    nc = bacc.Bacc(None, target_bir_lowering=False)

    x_d = nc.dram_tensor("x", [R, DIM], f32, kind="ExternalInput")
    # 7 scaled identities packed [P, KTAPS, P]
    wid_d = nc.dram_tensor("wident", [P, KTAPS, P], f32, kind="ExternalInput")
    srow_d = nc.dram_tensor("srow", [1, DIM], f16, kind="ExternalInput")
    drow_d = nc.dram_tensor("drow", [1, DIM], f16, kind="ExternalInput")
    bias_d = nc.dram_tensor("biasv", [1, 1], f32, kind="ExternalInput")
    slb_d = nc.dram_tensor("slbv", [1, 1], f32, kind="ExternalInput")
    ident_d = nc.dram_tensor("ident", [P, P], f32, kind="ExternalInput")
    zexp_d = nc.dram_tensor("zexp", [1, 1], f32, kind="ExternalInput")

    out_d = nc.dram_tensor("out", [R, DIM], f32, kind="ExternalOutput")
    ld_d = nc.dram_tensor("logdet", [R], f32, kind="ExternalOutput")

    def bcast(ap2d, parts):
        # AP that replays a [1, N] DRAM row across `parts` partitions
        return bass.AP(
            tensor=ap2d.tensor,
            offset=ap2d.offset,
            ap=[[0, parts]] + list(ap2d.ap[1:]),
        )

    with tile.TileContext(nc) as tc, ExitStack() as ctx:
        singles = ctx.enter_context(tc.tile_pool(name="singles", bufs=1))
        xsl_p = ctx.enter_context(tc.tile_pool(name="xsl", bufs=3))
        xslr_p = ctx.enter_context(tc.tile_pool(name="xslr", bufs=2))
        act_p = ctx.enter_context(tc.tile_pool(name="act", bufs=3))
        t_p = ctx.enter_context(tc.tile_pool(name="t", bufs=2))
        outb_p = ctx.enter_context(tc.tile_pool(name="outb", bufs=2))
        scr_p = ctx.enter_context(tc.tile_pool(name="scr", bufs=2))
        psum_p = ctx.enter_context(tc.tile_pool(name="psum", bufs=2, space="PSUM"))
        psum_ld_p = ctx.enter_context(tc.tile_pool(name="psum_ld", bufs=1, space="PSUM"))

        wid_raw = singles.tile([P, KTAPS, P], f32)
        nc.sync.dma_start(out=wid_raw, in_=wid_d[:])
        wid_sb = singles.tile([P, KTAPS, P], f32r)
        nc.scalar.copy(out=wid_sb, in_=wid_raw)
        scale_b = singles.tile([P, DIM], f16)
        nc.gpsimd.dma_start(out=scale_b, in_=bcast(srow_d[:], P))
        d_b = singles.tile([P, DIM], f16)
        nc.gpsimd.dma_start(out=d_b, in_=bcast(drow_d[:], P))
        bias_sb = singles.tile([P, 1], f32)
        nc.gpsimd.dma_start(out=bias_sb, in_=bcast(bias_d[:], P))
        slb_sb = singles.tile([P, 1], f32)
        nc.gpsimd.dma_start(out=slb_sb, in_=bcast(slb_d[:], P))
        zexp_sb = singles.tile([P, 1], f32)
        nc.gpsimd.dma_start(out=zexp_sb, in_=bcast(zexp_d[:], P))
        ident_sb = singles.tile([P, P], f32)
        nc.sync.dma_start(out=ident_sb, in_=ident_d[:])
        ldbuf = singles.tile([P, NT], f32)

        x_v = x_d[:].rearrange("(s j p) m -> s j p m", p=P, j=SLAB)
        out_v = out_d[:].rearrange("(s j p) m -> s j p m", p=P, j=SLAB)

        for s in range(NS):
            xsl = xsl_p.tile([P, SLAB, DIM], f32)
            nc.sync.dma_start(out=xsl, in_=x_v[s].rearrange("j p m -> p j m"))
            # fp32 -> fp32r producer for the PE
            xslr = xslr_p.tile([P, SLAB, DIM], f32r)
            nc.scalar.copy(out=xslr, in_=xsl)

            outb = outb_p.tile([P, SLAB, DIM], f32)
            for j in range(SLAB):
                ti = s * SLAB + j
                act = act_p.tile([P, DIM], f16)
                pss = [
                    psum_p.tile([P, 512], f32, name=f"ps{b}", tag=f"ps{b}")
                    for b in range(NBLK)
                ]
                for k in range(KTAPS):
                    for blk in range(NBLK):
                        c0 = blk * 512 + DIL * k
                        # clip to input width: features >= DIM are zero-padded,
                        # so taps contribute nothing to those output columns
                        w = min(512, DIM - c0)
                        nc.tensor.matmul(
                            pss[blk][:, 0:w],
                            wid_sb[:, k, :],
                            xslr[:, j, c0 : c0 + w],
                            start=(k == 0),
                            stop=(k == KTAPS - 1),
                            skip_group_check=True,
                        )
                if sim_compat:
                    # CoreSim lacks Lrelu: u = conv + bias, act = max(u, .01u)
                    u = act_p.tile([P, DIM], f16, name="u", tag="u")
                    for blk in range(NBLK):
                        nc.scalar.activation(
                            out=u[:, blk * 512 : (blk + 1) * 512],
                            in_=pss[blk],
                            func=mybir.ActivationFunctionType.Identity,
                            bias=bias_sb,
                            scale=1.0,
                        )
                    nc.vector.scalar_tensor_tensor(
                        out=act,
                        in0=u,
                        scalar=NEG,
                        in1=u,
                        op0=mybir.AluOpType.mult,
                        op1=mybir.AluOpType.max,
                    )
                else:
                    for blk in range(NBLK):
                        # act = lrelu(conv + bias), PSUM -> SBUF fp16
                        nc.scalar.activation(
                            out=act[:, blk * 512 : (blk + 1) * 512],
                            in_=pss[blk],
                            func=mybir.ActivationFunctionType.Lrelu,
                            bias=bias_sb,
                            scale=1.0,
                            alpha=NEG,
                        )

                t = t_p.tile([P, DIM], f16)
                nc.vector.tensor_tensor(
                    out=t, in0=act, in1=scale_b, op=mybir.AluOpType.mult
                )
                # out_f32 = t * 2^z + x   (restores the host-side scale shift)
                nc.vector.scalar_tensor_tensor(
                    out=outb[:, j, :],
                    in0=t,
                    scalar=zexp_sb,
                    in1=xsl[:, j, 0:DIM],
                    op0=mybir.AluOpType.mult,
                    op1=mybir.AluOpType.add,
                )
                scr = scr_p.tile([P, DIM], f16)
                # logdet tile contribution: sum_i (act>=0) * d_i
                nc.vector.scalar_tensor_tensor(
                    out=scr,
                    in0=act,
                    scalar=0.0,
                    in1=d_b,
                    op0=mybir.AluOpType.is_ge,
                    op1=mybir.AluOpType.mult,
                    accum_out=ldbuf[:, ti : ti + 1],
                )
            nc.scalar.dma_start(out=out_v[s].rearrange("j p m -> p j m"), in_=outb)

        # ---- logdet epilogue: [P, NT] -> [NT, P] via PE, add sum(lb), DMA out
        ld_ps = psum_ld_p.tile([P, P], f32)
        nc.tensor.transpose(ld_ps[:NT, :], ldbuf, ident_sb)
        ld_sb = singles.tile([NT, P], f32)
        nc.scalar.activation(
            out=ld_sb,
            in_=ld_ps[:NT, :],
            func=mybir.ActivationFunctionType.Identity,
            bias=slb_sb[:NT],
            scale=1.0,
        )
        nc.sync.dma_start(out=ld_d[:].rearrange("(t p) -> t p", p=P), in_=ld_sb)

    nc.compile()
    return nc


def _host_consts(weight, bias, lmbd):
    w = np.asarray(weight, dtype=np.float64).reshape(-1)
    b = float(np.asarray(bias).reshape(-1)[0])
    lm = np.asarray(lmbd, dtype=np.float64).reshape(-1)
    w0 = w[0]
    # stable softplus
    sp = np.logaddexp(0.0, lm)
    if w0 == 0.0:
        scale = lm.copy()
    elif w0 > 0:
        scale = -1.0 / w0 + sp
    else:
        scale = -1.0 / w0 - sp
    la = np.log(np.abs(scale * w0 + 1.0))
    lb = np.log(np.abs(NEG * scale * w0 + 1.0))
    d = la - lb
    sum_lb = lb.sum()

    # keep |scale/2^z| fp16-safe (< 2^14); restore via *2^z in the STT
    smax = float(np.abs(scale).max())
    z = max(0, int(math.ceil(math.log2(smax / 16384.0)))) if smax > 16384.0 else 0
    scale_scaled = scale / (2.0**z)

    wid = np.zeros((P, KTAPS, P), dtype=np.float32)
    eye = np.eye(P, dtype=np.float32)
    for k in range(KTAPS):
        wid[:, k, :] = np.float32(w[k]) * eye

    return {
        "wident": wid,
        "srow": scale_scaled.reshape(1, DIM).astype(np.float16),
        "drow": d.reshape(1, DIM).astype(np.float16),
        "biasv": np.array([[b]], dtype=np.float32),
        "slbv": np.array([[sum_lb]], dtype=np.float32),
        "ident": np.eye(P, dtype=np.float32),
        "zexp": np.array([[2.0**z]], dtype=np.float32),
    }


def kernel(x, weight, bias, lmbd):
    from concourse.bass_utils import run_bass_kernel_spmd

    x = np.ascontiguousarray(np.asarray(x, dtype=np.float32))
    consts = _host_consts(weight, bias, lmbd)

    if "nc" not in _PROGRAM_CACHE:
        _PROGRAM_CACHE["nc"] = _build_program()
    nc = _PROGRAM_CACHE["nc"]

    core_ids = list(range(N_CORES))
    in_maps = [{"x": x[i * R : (i + 1) * R], **consts} for i in range(N_CORES)]
    res = run_bass_kernel_spmd(nc, in_maps, core_ids).results
    out = np.concatenate([r["out"] for r in res], axis=0)
    logdet = np.concatenate([r["logdet"] for r in res], axis=0)
    return out.astype(np.float32), logdet.astype(np.float32)
